# revision 3
# baseline (speedup 1.0000x reference)
"""Trainium2 Bass kernel for a 2-layer GRU decoder step (B=32768, H=256).

Sharding: pure data parallel — batch split across 8 NeuronCores, weights
replicated. On-chip layout is feature-major ([H, B_core]); the host does the
transposes so the device pipeline (linear -> GRU1 -> GRU2) needs no on-chip
transposes at all: each matmul's PSUM output [M=features, N=batch] feeds the
next stage directly.
"""

import os
import sys

if "/opt/trn_rl_repo" not in sys.path:
    sys.path.insert(0, "/opt/trn_rl_repo")

import numpy as np

import concourse.bacc as bacc
import concourse.mybir as mybir
from concourse import bass, tile
from concourse.bass_utils import run_bass_kernel_spmd

N_CORES = 8
B = 32768
H = 256
BC = B // N_CORES  # batch rows per core

F32 = mybir.dt.float32
F32R = mybir.dt.float32r
AF = mybir.ActivationFunctionType
OP = mybir.AluOpType

# Tunables
BT = 512  # batch-tile (matmul moving dim; one fp32 PSUM bank)
CW = 512  # DMA chunk width (batch cols per streamed chunk)
MM_DT = F32R  # matmul input dtype
# engine for the SBUF-only tensor-tensor ops: "vector" or "gpsimd"
ENG_D = "vector"
ENG_U = "vector"

KT_CAT = 4  # 512 input features / 128
KT_H = 2  # 256 features / 128

_last_results = None  # stashed BassKernelResults (for the test harness)
_built = {}


def _mm(x):
    return x


def _f32(x):
    return x if MM_DT == F32 else x.bitcast(F32)


def _build():
    key = (BT, CW, MM_DT, ENG_D, ENG_U)
    if key in _built:
        return _built[key]

    nc = bacc.Bacc("TRN2", target_bir_lowering=False, debug=False)

    cat_d = nc.dram_tensor("cat_t", [2 * H, BC], MM_DT, kind="ExternalInput")
    h1_d = nc.dram_tensor("h1_t", [H, BC], MM_DT, kind="ExternalInput")
    h2_d = nc.dram_tensor("h2_t", [H, BC], MM_DT, kind="ExternalInput")
    wlin_d = nc.dram_tensor("wlin_t", [2 * H, H], MM_DT, kind="ExternalInput")
    wih1_d = nc.dram_tensor("wih1_t", [H, 3 * H], MM_DT, kind="ExternalInput")
    whh1_d = nc.dram_tensor("whh1_t", [H, 3 * H], MM_DT, kind="ExternalInput")
    wih2_d = nc.dram_tensor("wih2_t", [H, 3 * H], MM_DT, kind="ExternalInput")
    whh2_d = nc.dram_tensor("whh2_t", [H, 3 * H], MM_DT, kind="ExternalInput")
    brz1_d = nc.dram_tensor("brz1", [128, 4], F32, kind="ExternalInput")
    bin1_d = nc.dram_tensor("bin1", [128, 2], F32, kind="ExternalInput")
    bhn1_d = nc.dram_tensor("bhn1", [128, 2], F32, kind="ExternalInput")
    brz2_d = nc.dram_tensor("brz2", [128, 4], F32, kind="ExternalInput")
    bin2_d = nc.dram_tensor("bin2", [128, 2], F32, kind="ExternalInput")
    bhn2_d = nc.dram_tensor("bhn2", [128, 2], F32, kind="ExternalInput")
    xout_d = nc.dram_tensor("xout_t", [H, BC], F32, kind="ExternalOutput")
    h0o_d = nc.dram_tensor("h0o_t", [H, BC], F32, kind="ExternalOutput")
    h1o_d = nc.dram_tensor("h1o_t", [H, BC], F32, kind="ExternalOutput")

    def fm(ap, kt):  # DRAM [kt*128, N] -> feature-major [p, kt, N]
        return ap.ap().rearrange("(kt p) b -> p kt b", p=128)

    cat_a = fm(cat_d, KT_CAT)
    h1_a, h2_a = fm(h1_d, KT_H), fm(h2_d, KT_H)
    xout_a, h0o_a, h1o_a = fm(xout_d, KT_H), fm(h0o_d, KT_H), fm(h1o_d, KT_H)

    with tile.TileContext(nc) as tc:
        with (
            tc.tile_pool(name="wpool", bufs=1) as wp,
            tc.tile_pool(name="inpool", bufs=3) as ip,
            tc.tile_pool(name="xpool", bufs=2) as xp,
            tc.tile_pool(name="opool", bufs=2) as op_,
            tc.tile_pool(name="gates", bufs=4) as gp,
            tc.tile_pool(name="psum", bufs=8, space="PSUM") as pp,
        ):
            wlin_t = wp.tile([128, KT_CAT, H], MM_DT, tag="wlin")
            nc.sync.dma_start(wlin_t[:], fm(wlin_d, KT_CAT))
            wih1_t = wp.tile([128, KT_H, 3 * H], MM_DT, tag="wih1")
            nc.sync.dma_start(wih1_t[:], fm(wih1_d, KT_H))
            whh1_t = wp.tile([128, KT_H, 3 * H], MM_DT, tag="whh1")
            nc.sync.dma_start(whh1_t[:], fm(whh1_d, KT_H))
            wih2_t = wp.tile([128, KT_H, 3 * H], MM_DT, tag="wih2")
            nc.sync.dma_start(wih2_t[:], fm(wih2_d, KT_H))
            whh2_t = wp.tile([128, KT_H, 3 * H], MM_DT, tag="whh2")
            nc.sync.dma_start(whh2_t[:], fm(whh2_d, KT_H))
            bias = {}
            for nm, d, w in (
                ("brz1", brz1_d, 4),
                ("bin1", bin1_d, 2),
                ("bhn1", bhn1_d, 2),
                ("brz2", brz2_d, 4),
                ("bin2", bin2_d, 2),
                ("bhn2", bhn2_d, 2),
            ):
                t = wp.tile([128, w], F32, tag=nm)
                nc.sync.dma_start(t[:], d.ap())
                bias[nm] = t

            tt_eng = {"vector": nc.vector, "gpsimd": nc.gpsimd}

            def gru(x_in, h_in, wih, whh, brz, bin_, bhn, h_out, x_out, bs):
                # PSUM: r,z gates accumulate gi+gh; n parts kept separate
                rz_ps = []
                for mt in range(4):
                    ps = pp.tile([128, BT], F32, tag="ps")
                    ms = bass.ts(mt, 128)
                    for kt in range(KT_H):
                        nc.tensor.matmul(
                            ps[:],
                            _mm(wih[:, kt, ms]),
                            _mm(x_in[:, kt, bs]),
                            start=(kt == 0),
                            stop=False,
                        )
                    for kt in range(KT_H):
                        nc.tensor.matmul(
                            ps[:],
                            _mm(whh[:, kt, ms]),
                            _mm(h_in[:, kt, bs]),
                            start=False,
                            stop=(kt == KT_H - 1),
                        )
                    rz_ps.append(ps)
                in_ps, hn_ps = [], []
                for ft in range(2):
                    ms = bass.ts(4 + ft, 128)
                    ps = pp.tile([128, BT], F32, tag="ps")
                    for kt in range(KT_H):
                        nc.tensor.matmul(
                            ps[:],
                            _mm(wih[:, kt, ms]),
                            _mm(x_in[:, kt, bs]),
                            start=(kt == 0),
                            stop=(kt == KT_H - 1),
                        )
                    in_ps.append(ps)
                    ps = pp.tile([128, BT], F32, tag="ps")
                    for kt in range(KT_H):
                        nc.tensor.matmul(
                            ps[:],
                            _mm(whh[:, kt, ms]),
                            _mm(h_in[:, kt, bs]),
                            start=(kt == 0),
                            stop=(kt == KT_H - 1),
                        )
                    hn_ps.append(ps)
                for ft in range(2):
                    r = gp.tile([128, BT], F32, tag="r")
                    nc.scalar.activation(
                        r[:], rz_ps[ft][:], AF.Sigmoid, bias=brz[:, ft : ft + 1]
                    )
                    z = gp.tile([128, BT], F32, tag="z")
                    nc.scalar.activation(
                        z[:], rz_ps[2 + ft][:], AF.Sigmoid, bias=brz[:, 2 + ft : 3 + ft]
                    )
                    t = gp.tile([128, BT], F32, tag="t")
                    # t = (h_n + bhn) * r
                    nc.vector.scalar_tensor_tensor(
                        t[:], hn_ps[ft][:], bhn[:, ft : ft + 1], r[:], OP.add, OP.mult
                    )
                    t2 = gp.tile([128, BT], F32, tag="t2")
                    # t2 = (i_n + bin) + t
                    nc.vector.scalar_tensor_tensor(
                        t2[:], in_ps[ft][:], bin_[:, ft : ft + 1], t[:], OP.add, OP.add
                    )
                    n = gp.tile([128, BT], F32, tag="n")
                    nc.scalar.activation(n[:], t2[:], AF.Tanh)
                    d = gp.tile([128, BT], F32, tag="d")
                    tt_eng[ENG_D].tensor_tensor(d[:], _f32(h_in[:, ft, bs]), n[:], OP.subtract)
                    u = gp.tile([128, BT], F32, tag="u")
                    tt_eng[ENG_U].tensor_tensor(u[:], z[:], d[:], OP.mult)
                    # h' = n + u ; x' = x + h'
                    nc.vector.tensor_tensor(h_out[:, ft, bs], n[:], u[:], OP.add)
                    nc.vector.tensor_tensor(
                        x_out[:, ft, bs], _f32(x_in[:, ft, bs]), h_out[:, ft, bs], OP.add
                    )

            for c in range(BC // CW):
                cs = bass.ts(c, CW)
                cat_t = ip.tile([128, KT_CAT, CW], MM_DT, tag="cat")
                nc.sync.dma_start(cat_t[:], cat_a[:, :, cs])
                h1_t = ip.tile([128, KT_H, CW], MM_DT, tag="h1")
                nc.sync.dma_start(h1_t[:], h1_a[:, :, cs])
                h2_t = ip.tile([128, KT_H, CW], MM_DT, tag="h2")
                nc.sync.dma_start(h2_t[:], h2_a[:, :, cs])
                xlin_t = xp.tile([128, KT_H, CW], MM_DT, tag="xlin")
                x1_t = xp.tile([128, KT_H, CW], MM_DT, tag="x1")
                xout_t = op_.tile([128, KT_H, CW], F32, tag="xout")
                h0o_t = op_.tile([128, KT_H, CW], F32, tag="h0o")
                h1o_t = op_.tile([128, KT_H, CW], F32, tag="h1o")
                for j in range(CW // BT):
                    bs = bass.ts(j, BT)
                    # linear: x = W_lin @ cat (no bias in reference)
                    for mt in range(KT_H):
                        ps = pp.tile([128, BT], F32, tag="ps")
                        ms = bass.ts(mt, 128)
                        for kt in range(KT_CAT):
                            nc.tensor.matmul(
                                ps[:],
                                _mm(wlin_t[:, kt, ms]),
                                _mm(cat_t[:, kt, bs]),
                                start=(kt == 0),
                                stop=(kt == KT_CAT - 1),
                            )
                        nc.scalar.copy(xlin_t[:, mt, bs], ps[:])
                    gru(
                        xlin_t, h1_t, wih1_t, whh1_t,
                        bias["brz1"], bias["bin1"], bias["bhn1"],
                        h0o_t, x1_t, bs,
                    )
                    gru(
                        x1_t, h2_t, wih2_t, whh2_t,
                        bias["brz2"], bias["bin2"], bias["bhn2"],
                        h1o_t, xout_t, bs,
                    )
                nc.sync.dma_start(xout_a[:, :, cs], xout_t[:])
                nc.sync.dma_start(h0o_a[:, :, cs], h0o_t[:])
                nc.sync.dma_start(h1o_a[:, :, cs], h1o_t[:])

    nc.compile()
    _built[key] = nc
    return nc


def _bias_fm(b):  # [k*128] -> [128, k] feature-major
    return np.ascontiguousarray(b.reshape(-1, 128).T)


def kernel(
    attn_out,
    attn_rnn_hidden,
    dec_rnn_hiddens,
    W_lin,
    gru1_Wih,
    gru1_Whh,
    gru1_bih,
    gru1_bhh,
    gru2_Wih,
    gru2_Whh,
    gru2_bih,
    gru2_bhh,
):
    global _last_results
    f = np.float32
    catT = np.empty((2 * H, B), dtype=f)
    catT[:H] = attn_rnn_hidden.T
    catT[H:] = attn_out.T
    h1T = np.ascontiguousarray(dec_rnn_hiddens[0].T, dtype=f)
    h2T = np.ascontiguousarray(dec_rnn_hiddens[1].T, dtype=f)
    shared = {
        "wlin_t": np.ascontiguousarray(W_lin.T, dtype=f),
        "wih1_t": np.ascontiguousarray(gru1_Wih.T, dtype=f),
        "whh1_t": np.ascontiguousarray(gru1_Whh.T, dtype=f),
        "wih2_t": np.ascontiguousarray(gru2_Wih.T, dtype=f),
        "whh2_t": np.ascontiguousarray(gru2_Whh.T, dtype=f),
        "brz1": _bias_fm((gru1_bih + gru1_bhh)[: 2 * H].astype(f)),
        "bin1": _bias_fm(gru1_bih[2 * H :].astype(f)),
        "bhn1": _bias_fm(gru1_bhh[2 * H :].astype(f)),
        "brz2": _bias_fm((gru2_bih + gru2_bhh)[: 2 * H].astype(f)),
        "bin2": _bias_fm(gru2_bih[2 * H :].astype(f)),
        "bhn2": _bias_fm(gru2_bhh[2 * H :].astype(f)),
    }
    in_maps = []
    for c in range(N_CORES):
        s = slice(c * BC, (c + 1) * BC)
        m = dict(shared)
        m["cat_t"] = np.ascontiguousarray(catT[:, s])
        m["h1_t"] = np.ascontiguousarray(h1T[:, s])
        m["h2_t"] = np.ascontiguousarray(h2T[:, s])
        in_maps.append(m)

    nc = _build()
    res = run_bass_kernel_spmd(nc, in_maps, core_ids=list(range(N_CORES)))
    _last_results = res

    xT = np.empty((H, B), dtype=f)
    h0T = np.empty((H, B), dtype=f)
    h1oT = np.empty((H, B), dtype=f)
    for c in range(N_CORES):
        s = slice(c * BC, (c + 1) * BC)
        xT[:, s] = res.results[c]["xout_t"]
        h0T[:, s] = res.results[c]["h0o_t"]
        h1oT[:, s] = res.results[c]["h1o_t"]
    x = np.ascontiguousarray(xT.T)
    hiddens = np.stack(
        [np.ascontiguousarray(h0T.T), np.ascontiguousarray(h1oT.T)], axis=0
    )
    return x, hiddens
